# revision 1
# baseline (speedup 1.0000x reference)
"""AdaProj loss kernel for 8 TRN2 NeuronCores (Bass/Tile), v5.

Math (per reference):
  xn = l2norm(x, 1); Wn = l2norm(W, 2); coef[b,c,s] = xn . Wn[c,s]
  q1 = sum_s coef^2 ; q2 = coef^T G_c coef (G = Wn Wn^T)
  logits = q1/sqrt(q2); loss = mean_b( lse_c(s*logits) - s*logits[b,lab] )

All normalizations are algebraically folded; with RAW inputs
(craw = W-rows . x, rsq = |W row|^2, rinv = 1/rsq, v = rinv*craw):
  q1r = sum_s v*craw ; q2r = sum_s v*(Graw v) ; logits = q1r/sqrt(q2r*|x|^2)

Sharding: class-parallel, C=512 -> 64 classes/core; W and x shards are sent
host-transposed (wT [E,CS], xT [E,B]). Each core returns
  out[0,:] = sum_{c in shard} exp(s*logits - s) ; out[1,:] = sum_c y*logits
Host: loss = mean( log(sum_i se_i) + s - s*sum_i t0_i ).

v5: software-pipelined main loop. Per-iteration structure:
  prologue: W/x DMA + bf16 casts; all 16 class-tile Grams (PE) packed into
    2 psum tiles, evacuated to SBUF (Act); per-tile diag->rsq/rinv (DVE),
    masked Gram blocks (DVE); |x|^2 via ones-matmul + 64-row bcast DMA.
  steps s=0..17 (stagger): PE: reduce(s-2) -> craw(s) -> h(s-1);
    Act: vp(s-1) = rinv*craw psum evac; DVE: ep(s-1), chp(s-1).
  PSUM: qps [128,1024] accum (2 banks) + pc 2x[128,1024] (4) + ph 2x[128,512] (2).
"""

import sys

for _p in ("/opt/trn_rl_repo",):
    if _p not in sys.path:
        sys.path.insert(0, _p)

import ml_dtypes
import numpy as np

import bass_rust
import concourse.bass as bass
import concourse.tile as tile
from concourse import mybir
from concourse.bass_utils import run_bass_kernel_spmd

FP32 = mybir.dt.float32
BF16 = mybir.dt.bfloat16
FP8 = mybir.dt.float8e4

# fp8 ep/chp + DoubleRow pair-reduce matmuls (2x PE on reduces); falls back
# to bf16 ep/chp + per-tile bf16 reduces when False.
USE_FP8 = True

# route h through an Act psum-evac for even tiles (chp becomes an
# all-SBUF 2x DVE TT; the missing ris factor rides in indw)
HSB = True

B, C, S, E = 1024, 512, 32, 128
NCORES = 8
C_LOC = C // NCORES            # 64 classes per core
CS = C_LOC * S                 # 2048 rows of the local basis
NT = CS // 128                 # 16 cs-tiles of 128 rows (4 classes each)
NB = B // 512                  # psum-bank chunks of the batch
CPT = 128 // S                 # classes per cs-tile = 4


def build_nc(s_val: float, n_iters: int = 1, hw_loop: bool = False) -> bass.Bass:
    nc = bass.Bass()

    w_ext = nc.declare_dram_parameter("wT", [E, CS], BF16, isOutput=False)
    x_ext = nc.declare_dram_parameter("xT", [E, B], BF16, isOutput=False)
    yt_ext = nc.declare_dram_parameter("yt", [C_LOC, B], BF16, isOutput=False)
    mask_ext = nc.declare_dram_parameter("mask", [128, 512], BF16, isOutput=False)
    id_ext = nc.declare_dram_parameter("ident", [128, 128], FP32, isOutput=False)
    ind_ext = nc.declare_dram_parameter("ind2", [128, 1024], FP8, isOutput=False)
    indp_ext = nc.declare_dram_parameter("indp", [128, 124], BF16, isOutput=False)
    ones_ext = nc.declare_dram_parameter("ones", [128, 1], BF16, isOutput=False)
    out_ext = nc.declare_dram_parameter("out", [2, B], FP32, isOutput=True)

    Mult = mybir.AluOpType.mult
    Exp = mybir.ActivationFunctionType.Exp
    Ln = mybir.ActivationFunctionType.Ln
    Copy = mybir.ActivationFunctionType.Copy

    with tile.TileContext(nc) as tc:
        with (
            tc.tile_pool(name="persist", bufs=1) as pp,
            tc.tile_pool(name="xload", bufs=2) as p_x,
            tc.tile_pool(name="wload", bufs=2) as p_w,
            tc.tile_pool(name="gram", bufs=2) as p_g,
            tc.tile_pool(name="stream", bufs=3) as p_s,
            tc.tile_pool(name="fin", bufs=2) as p_f,
            tc.tile_pool(name="psC", bufs=2, space="PSUM") as ps_c,
            tc.tile_pool(name="psH", bufs=2, space="PSUM") as ps_h,
            tc.tile_pool(name="psQ", bufs=1, space="PSUM") as ps_q,
        ):
            # ---- constants, loaded once
            mask = pp.tile([128, 512], BF16, tag="mask")
            nc.sync.dma_start(out=mask[:], in_=mask_ext[:])
            ident = pp.tile([128, 128], FP32, tag="ident")
            nc.sync.dma_start(out=ident[:], in_=id_ext[:])
            ind2 = pp.tile([128, 1024], FP8, tag="ind2")
            nc.sync.dma_start(out=ind2[:], in_=ind_ext[:])
            indp = pp.tile([128, 124], BF16, tag="indp")
            nc.sync.dma_start(out=indp[:], in_=indp_ext[:])
            ones = pp.tile([128, 1], BF16, tag="ones")
            nc.sync.dma_start(out=ones[:], in_=ones_ext[:])
            yt = pp.tile([C_LOC, B], BF16, tag="yt")
            nc.sync.dma_start(out=yt[:], in_=yt_ext[:])
            negs = pp.tile([128, 1], FP32, tag="negs")
            nc.vector.memset(negs[:], -s_val)

            def body():
                # ---- loads: W and x arrive host-cast to bf16 (the device
                # only ever consumed them as bf16; saves the casts + half
                # the DMA bytes)
                wTb = p_w.tile([128, CS], BF16, tag="wTb")
                nc.sync.dma_start(out=wTb[:], in_=w_ext[:])

                xTb = p_x.tile([128, B], BF16, tag="xTb")
                nc.sync.dma_start(out=xTb[:], in_=x_ext[:])

                # ---- all 16 Gram tiles, packed 8 per psum buffer
                gAll = p_g.tile([128, CS], FP32, tag="gAll")
                for half in range(2):
                    pg = ps_c.tile([128, B], FP32, tag="pc")
                    for j in range(8):
                        t = 8 * half + j
                        wsl = wTb[:, t * 128 : (t + 1) * 128]
                        nc.tensor.matmul(
                            pg[:, j * 128 : (j + 1) * 128], lhsT=wsl, rhs=wsl,
                            start=True, stop=True,
                        )
                    nc.scalar.copy(
                        out=gAll[:, half * B : (half + 1) * B], in_=pg[:]
                    )

                # ---- |x|^2 per column: square, ones-matmul, bcast to rows 64:128
                sqx = p_x.tile([128, B], BF16, tag="sqx")
                nc.gpsimd.tensor_tensor(
                    out=sqx[:], in0=xTb[:], in1=xTb[:], op=Mult
                )
                pxs = ps_h.tile([128, 512], FP32, tag="ph")
                pxs2 = ps_h.tile([128, 512], FP32, tag="ph")
                nc.tensor.matmul(pxs[0:1, :], lhsT=ones[:], rhs=sqx[:, 0:512],
                                 start=True, stop=True)
                nc.tensor.matmul(pxs2[0:1, :], lhsT=ones[:], rhs=sqx[:, 512:1024],
                                 start=True, stop=True)
                xssr = p_x.tile([1, B], FP32, tag="xssr")
                nc.scalar.copy(out=xssr[:, 0:512], in_=pxs[0:1, :])
                nc.scalar.copy(out=xssr[:, 512:1024], in_=pxs2[0:1, :])
                xssB = p_x.tile([128, B], FP32, tag="xssB")
                _src = xssr[0:1, :]
                nc.sync.dma_start(
                    out=xssB[64:128, :],
                    in_=bass.AP(
                        tensor=_src.tensor, offset=_src.offset,
                        ap=[[1, 1], [0, 64], [1, B]],
                    ),
                )

                # rsq per cs-row via PE: rsq = (w (.) w)^T @ ones; then
                # ris = rsq^-1/2 (Ln/Exp, same act table as the finals);
                # gb = G (.) mask (.) ris[contraction row] so that
                # h = gb^T vp = G (rinv craw) exactly when vp = ris craw.
                wsl2 = p_g.tile([128, CS], BF16, tag="wsl2")
                nc.vector.tensor_tensor(
                    out=wsl2[:], in0=wTb[:], in1=wTb[:], op=Mult
                )
                psr = ps_h.tile([128, 512], FP32, tag="ph")
                for t in range(NT):
                    nc.tensor.matmul(
                        psr[:, t : t + 1],
                        lhsT=wsl2[:, t * 128 : (t + 1) * 128],
                        rhs=ones[:], start=True, stop=True,
                    )
                rsq = p_g.tile([128, NT], FP32, tag="rsq")
                nc.vector.tensor_copy(rsq[:], psr[:, 0:NT])
                ris = p_g.tile([128, NT], FP32, tag="ris")
                nc.scalar.activation(out=ris[:], in_=rsq[:], func=Ln)
                nc.scalar.activation(out=ris[:], in_=ris[:], func=Exp, scale=-0.5)
                gb = p_g.tile([128, CS], BF16, tag="gb")

                def emit_gb(t):
                    nc.vector.scalar_tensor_tensor(
                        out=gb[:, t * 128 : (t + 1) * 128],
                        in0=gAll[:, t * 128 : (t + 1) * 128],
                        scalar=ris[:, t : t + 1],
                        in1=mask[:, 0:128], op0=Mult, op1=Mult,
                    )

                # only the first 4 gb tiles upfront; the rest interleave into
                # the step loop (6 tiles ahead of their h-matmul) so DVE's
                # early-iteration queue reaches ep/chp before PE's first
                # reduce needs them
                for t in range(4):
                    emit_gb(t)
                # ris-scaled q2 indicators for the Act-evac tiles (chp there is
                # a plain TT missing one ris factor; fold it into the reduce)
                indw = p_g.tile([128, 64 * NT], BF16, tag="indw")

                def emit_indw(t):
                    if HSB and t % 2 == 0:
                        nc.scalar.activation(
                            out=indw[:, 64 * t : 64 * (t + 1)],
                            in_=indp[:, 60 - CPT * t : 124 - CPT * t],
                            func=Copy, scale=ris[:, t : t + 1],
                        )

                # ---- software-pipelined main loop over cs-tiles
                # step s: PE: red_pair((s-3)//2 at odd s), craw(s), h(s-1);
                #         Act: vp(s-1); Pool: ep(s-1); DVE: chp(s-1)
                # ep/chp stored fp8 per tile-PAIR; reduce via DoubleRow
                # matmuls contracting both tiles of a pair at once.
                qps = ps_q.tile([128, B], FP32, tag="q")
                pcs, phs, vps, eps, chps = {}, {}, {}, {}, {}

                def emit_craw(t):
                    pc = ps_c.tile([128, B], FP32, tag="pc")
                    wsl = wTb[:, t * 128 : (t + 1) * 128]
                    for nb in range(NB):
                        nc.tensor.matmul(
                            pc[:, nb * 512 : (nb + 1) * 512], lhsT=wsl,
                            rhs=xTb[:, nb * 512 : (nb + 1) * 512],
                            start=True, stop=True,
                        )
                    pcs[t] = pc

                hsbs = {}

                def emit_h(t):
                    vp = vps[t]
                    for nb in range(NB):
                        ph = ps_h.tile([128, 512], FP32, tag="ph")
                        nc.tensor.matmul(
                            ph[:], lhsT=gb[:, t * 128 : (t + 1) * 128],
                            rhs=vp[:, nb * 512 : (nb + 1) * 512],
                            start=True, stop=True,
                        )
                        phs[(t, nb)] = ph
                    if HSB and t % 2 == 0:
                        # Act-evac path: h to SBUF so chp is an all-SBUF TT
                        # (2x DVE mode); the missing ris factor rides in indw.
                        hsb = p_s.tile([128, B], BF16, tag="hsb", name="hsb")
                        for nb in range(NB):
                            nc.scalar.copy(
                                out=hsb[:, nb * 512 : (nb + 1) * 512],
                                in_=phs[(t, nb)][:],
                            )
                        hsbs[t] = hsb

                EDT = FP8 if USE_FP8 else BF16

                def emit_vp(t):
                    # vp2 = rsq^-1/2 craw, so ep = vp2 (.) vp2 needs no scalar
                    # (Pool tensor_tensor) and h = gb^T vp2 = G (rinv craw).
                    vp = p_s.tile([128, B], BF16, tag="vp")
                    nc.scalar.activation(
                        out=vp[:], in_=pcs[t][:], func=Copy,
                        scale=ris[:, t : t + 1],
                    )
                    vps[t] = vp

                def emit_ep(t):
                    if t % 2 == 0:
                        eps[t // 2] = p_s.tile(
                            [128, 2 * B], EDT, tag="ep", name="ep2"
                        )
                    ep = eps[t // 2]
                    nc.vector.tensor_tensor(
                        out=ep[:, (t % 2) * B : (t % 2 + 1) * B],
                        in0=vps[t][:], in1=vps[t][:], op=Mult,
                    )

                def emit_chp(t):
                    if t % 2 == 0:
                        chps[t // 2] = p_s.tile(
                            [128, 2 * B], BF16, tag="chp", name="chp2"
                        )
                    chp = chps[t // 2]
                    if HSB and t % 2 == 0:
                        nc.vector.tensor_tensor(
                            out=chp[:, (t % 2) * B : (t % 2 + 1) * B],
                            in0=vps[t][:], in1=hsbs[t][:], op=Mult,
                        )
                        return
                    for nb in range(NB):
                        nc.vector.scalar_tensor_tensor(
                            out=chp[:, (t % 2) * B + nb * 512 : (t % 2) * B + (nb + 1) * 512],
                            in0=vps[t][:, nb * 512 : (nb + 1) * 512],
                            scalar=ris[:, t : t + 1],
                            in1=phs[(t, nb)][:], op0=Mult, op1=Mult,
                        )

                def _pair_ap(tile2, nb):
                    # [128 part, 2 k-tiles, 512 cols] view of a [128, 2B] pair tile
                    src = tile2[:, nb * 512 : nb * 512 + 512]
                    return bass.AP(
                        tensor=src.tensor, offset=src.offset,
                        ap=[list(src.ap[0]), [B, 2], [1, 512]],
                    )

                def _ind_ap(u):
                    src = ind2[:, 128 * u : 128 * u + 64]
                    return bass.AP(
                        tensor=src.tensor, offset=src.offset,
                        ap=[list(src.ap[0]), [64, 2], [1, 64]],
                    )

                DR = mybir.MatmulPerfMode.DoubleRow

                def emit_red_pair(u):
                    # q1: fp8 DoubleRow over both tiles of the pair (out base 0
                    # only -- walrus ISA check rejects DoubleRow at base 64).
                    # q2: per-tile bf16 reduce at base 64.
                    for nb in range(NB):
                        if USE_FP8:
                            nc.tensor.matmul(
                                qps[0:64, nb * 512 : (nb + 1) * 512],
                                lhsT=_ind_ap(u),
                                rhs=_pair_ap(eps[u], nb),
                                start=(u == 0), stop=(u == NT // 2 - 1),
                                skip_group_check=True, perf_mode=DR,
                            )
                        for i in range(2):
                            t = 2 * u + i
                            ind_t = indp[:, 60 - CPT * t : 124 - CPT * t]
                            o = i * B + nb * 512
                            if not USE_FP8:
                                nc.tensor.matmul(
                                    qps[0:64, nb * 512 : (nb + 1) * 512],
                                    lhsT=ind_t, rhs=eps[u][:, o : o + 512],
                                    start=(t == 0), stop=(t == NT - 1),
                                    skip_group_check=True,
                                )
                            ind_q2 = (
                                indw[:, 64 * t : 64 * (t + 1)]
                                if (HSB and t % 2 == 0) else ind_t
                            )
                            nc.tensor.matmul(
                                qps[64:128, nb * 512 : (nb + 1) * 512],
                                lhsT=ind_q2, rhs=chps[u][:, o : o + 512],
                                start=(t == 0), stop=(t == NT - 1),
                                skip_group_check=True,
                            )

                for s in range(NT + 5):
                    if s < NT:
                        emit_indw(s)
                    if s + 4 < NT:
                        emit_gb(s + 4)
                    if s >= 5 and s % 2 == 1:
                        emit_red_pair((s - 5) // 2)
                    if s < NT:
                        emit_craw(s)
                    if 1 <= s <= NT:
                        emit_vp(s - 1)
                    if 2 <= s <= NT + 1:
                        emit_h(s - 2)
                        emit_ep(s - 2)
                        emit_chp(s - 2)

                # ---- logits, softmax partials, output (read qps psum directly)
                q2s = p_f.tile([C_LOC, B], FP32, tag="q2s")
                nc.vector.tensor_tensor(
                    out=q2s[:], in0=qps[64:128, :], in1=xssB[64:128, :], op=Mult
                )
                invs = p_f.tile([C_LOC, B], FP32, tag="invs")
                nc.scalar.activation(out=invs[:], in_=q2s[:], func=Ln)
                nc.scalar.activation(out=invs[:], in_=invs[:], func=Exp, scale=-0.5)
                logitsT = p_f.tile([C_LOC, B], BF16, tag="logitsT")
                nc.vector.tensor_tensor(
                    out=logitsT[:], in0=qps[0:64, :], in1=invs[:], op=Mult
                )
                expz = p_f.tile([C_LOC, B], BF16, tag="expz")
                nc.scalar.activation(
                    out=expz[:], in_=logitsT[:], func=Exp, scale=s_val,
                    bias=negs[0:C_LOC, :],
                )
                tl = p_f.tile([C_LOC, B], BF16, tag="tl")
                nc.vector.tensor_tensor(
                    out=tl[:], in0=yt[:], in1=logitsT[:], op=Mult
                )

                outse = p_f.tile([1, B], FP32, tag="outse")
                outt0 = p_f.tile([1, B], FP32, tag="outt0")
                for nb in range(NB):
                    pse = ps_h.tile([128, 512], FP32, tag="ph")
                    nc.tensor.matmul(
                        pse[0:1, :], lhsT=ones[0:C_LOC, :],
                        rhs=expz[:, nb * 512 : (nb + 1) * 512],
                        start=True, stop=True,
                    )
                    nc.scalar.copy(
                        out=outse[:, nb * 512 : (nb + 1) * 512], in_=pse[0:1, :]
                    )
                for nb in range(NB):
                    pt0 = ps_h.tile([128, 512], FP32, tag="ph")
                    nc.tensor.matmul(
                        pt0[0:1, :], lhsT=ones[0:C_LOC, :],
                        rhs=tl[:, nb * 512 : (nb + 1) * 512],
                        start=True, stop=True,
                    )
                    nc.scalar.copy(
                        out=outt0[:, nb * 512 : (nb + 1) * 512], in_=pt0[0:1, :]
                    )
                nc.sync.dma_start(out=out_ext[0:1, :], in_=outse[:])
                nc.sync.dma_start(out=out_ext[1:2, :], in_=outt0[:])

            if hw_loop:
                with tc.For_i(0, n_iters, 1):
                    body()
            else:
                for _ in range(n_iters):
                    body()

    # Split multi-wait sync_info into EventSemaphore instructions (HW allows
    # only 1 wait per instruction in this toolchain's walrus).
    bass_rust.move_matmul_waits_to_ldweights(nc.m)
    bass_rust.generate_event_semaphores(nc)
    return nc


def make_aux():
    mask = np.zeros((128, 512), dtype=ml_dtypes.bfloat16)
    for q in range(4):
        for j in range(CPT):
            mask[j * S : (j + 1) * S, q * 128 + j * S : q * 128 + (j + 1) * S] = 1.0
    ident = np.eye(128, dtype=np.float32)
    # ind2[k, 128u + 64i + m] = 1 iff class m == 4*(2u+i) + k//S
    # (per-pair two-slot indicator for DoubleRow reduce matmuls)
    ind2 = np.zeros((128, 1024), dtype=ml_dtypes.float8_e4m3)
    for k in range(128):
        for u in range(8):
            for i in range(2):
                ind2[k, 128 * u + 64 * i + 4 * (2 * u + i) + k // S] = 1.0
    indp = np.zeros((128, 124), dtype=ml_dtypes.bfloat16)
    for k in range(128):
        indp[k, 60 + k // S] = 1.0
    ones = np.ones((128, 1), dtype=ml_dtypes.bfloat16)
    return mask, ident, ind2, indp, ones


def make_in_maps(x, y, W):
    mask, ident, ind2, indp, ones = make_aux()
    xT = np.ascontiguousarray(x.T).astype(ml_dtypes.bfloat16)
    in_maps = []
    for i in range(NCORES):
        wT_i = np.ascontiguousarray(
            W[i * C_LOC : (i + 1) * C_LOC].reshape(CS, E).T
        ).astype(ml_dtypes.bfloat16)
        yt_i = np.ascontiguousarray(
            y[:, i * C_LOC : (i + 1) * C_LOC].T
        ).astype(ml_dtypes.bfloat16)
        in_maps.append(
            {
                "wT": wT_i, "xT": xT, "yt": yt_i,
                "mask": mask, "ident": ident, "ind2": ind2, "indp": indp,
                "ones": ones,
            }
        )
    return in_maps


def combine(outs, s_val):
    se = np.zeros(B, dtype=np.float64)
    t0 = np.zeros(B, dtype=np.float64)
    for o in outs:
        se += o[0]
        t0 += o[1]
    return np.float32(np.mean(np.log(se) + s_val - s_val * t0))


_CACHE = {}


def kernel(x, y, W, s, **_unused):
    x = np.ascontiguousarray(np.asarray(x, dtype=np.float32))
    y = np.asarray(y, dtype=np.float32)
    W = np.asarray(W, dtype=np.float32)
    s_val = float(np.asarray(s))

    key = ("v5", s_val)
    nc = _CACHE.get(key)
    if nc is None:
        nc = build_nc(s_val)
        _CACHE[key] = nc

    in_maps = make_in_maps(x, y, W)
    res = run_bass_kernel_spmd(nc, in_maps, core_ids=list(range(NCORES)))
    outs = [np.asarray(r["out"], dtype=np.float64) for r in res.results]
    return combine(outs, s_val)


if __name__ == "__main__":
    rng = np.random.default_rng(0)
    x = rng.standard_normal((B, E), dtype=np.float32)
    lab = rng.integers(0, C, size=B)
    y = np.eye(C, dtype=np.float32)[lab]
    W = rng.uniform(-0.1, 0.1, size=(C, S, E)).astype(np.float32)
    s = np.float32(np.sqrt(2.0) * np.log(C - 1.0))
    print(kernel(x=x, y=y, W=W, s=s))



# revision 24
# speedup vs baseline: 1.4400x; 1.4400x over previous
"""AdaProj loss kernel for 8 TRN2 NeuronCores (Bass/Tile), v7.

Math (per reference):
  xn = l2norm(x, 1); Wn = l2norm(W, 2)  [C,S,E]
  q1 = |Wn_c x|^2 ; q2 = (Wn_c x)^T G_c (Wn_c x), G_c = Wn_c Wn_c^T
  logits = q1/sqrt(q2*|x|^2); loss = mean_b( lse_c(s*logits) - s*logits[b,lab] )

v7: host precomputes Wn and the Cholesky G_c = L_c L_c^T, so
  q2 = |M_c x|^2 with M_c = L_c^T Wn_c.  Both quadratic forms become
  squared linear projections of x. Host stacks A = interleave(Wn, M) into
  32 cs-tiles of 128 rows (even tile 2g: Wn rows of classes 4g..4g+3,
  odd tile 2g+1: M rows of the same classes).

Device per tile t: one matmul t_ps = A_t^T x  [128, B] psum; one fused
square-evacuation psum -> fp8 SBUF (Act activation(Square) or DVE
tensor_tensor mult, alternating for engine balance). Pairs (2g, 2g+1)
land in one [128, 2B] fp8 buffer = (ep | up); a single fp8 DoubleRow
indicator matmul per pair accumulates q1 into psum rows 0:64 and q2
into rows 64:128 of the same psum tile.

|x|^2 comes from the host (xss broadcast [64, B] bf16).

The finals (logits -> softmax partials) are software-pipelined ACROSS
iterations: each body ends with qps -> qcopy (persistent SBUF); the
finals chain reading qcopy is emitted interleaved into the NEXT body's
tile loop (and once after the loop for the last iteration), so the
serial logits tail hides under the next iteration's matmul/square work.

Sharding: class-parallel, C=512 -> 64 classes/core. Each core returns
  out[0,:] = sum_{c in shard} exp(s*logits - s) ; out[1,:] = sum_c y*logits
Host: loss = mean( log(sum_i se_i) + s - s*sum_i t0_i ).
"""

import sys

for _p in ("/opt/trn_rl_repo",):
    if _p not in sys.path:
        sys.path.insert(0, _p)

import ml_dtypes
import numpy as np

import bass_rust
import concourse.bass as bass
import concourse.tile as tile
from concourse import mybir
from concourse.bass_utils import run_bass_kernel_spmd

FP32 = mybir.dt.float32
BF16 = mybir.dt.bfloat16
FP8 = mybir.dt.float8e4

B, C, S, E = 1024, 512, 32, 128
NCORES = 8
C_LOC = C // NCORES            # 64 classes per core
NG = C_LOC // 4                # 16 groups of 4 classes
NT = 2 * NG                    # 32 cs-tiles (even: Wn/q1, odd: M/q2)
NB = B // 512                  # psum-bank chunks of the batch

# square-evac mode per tile:
#   'A': Act activation(Square) psum->fp8 (fused evac+square)
#   'V': DVE copy psum->bf16, then DVE TT square ->fp8 (keeps fp8 DR pair)
#   'P': DVE copy psum->bf16, then Pool TT square ->bf16 (single bf16 reduce)
# (DVE cannot square from PSUM: only one PSUM operand per instruction.)
# per-PAIR modes, interleaved so the Act/DVE/Pool square streams overlap
# (a bunched P-stretch serializes on the slow Pool TT)
import os as _os
_PAIRS = (_os.environ.get("V7_PAIRS") or
          "AA,PP,AA,PP,AA,AV,PP,AA,PP,AA,VV,PP,AA,AA,AA,AA").split(",")
MODE = [m for p in _PAIRS for m in p]
assert len(MODE) == NT
# pairs where both tiles produce fp8 use one DoubleRow reduce; others get
# two single-tile reduces through the sliding-window indicator indb
DR_PAIR = [MODE[2 * g] != 'P' and MODE[2 * g + 1] != 'P' for g in range(NG)]

# tile steps at which the 6 pipelined finals ops are emitted
FIN_STEPS = (6, 10, 14, 18, 22, 26)
# defer the from-SBUF squares (V: DVE, P: Pool) this many steps after the
# evacuating copy, so they never delay the PSUM-freeing copies behind them
SQ_LAG = 3
# DMA the output straight from PSUM (no SBUF bounce) — rejected by this
# toolchain (dma_start requires SBUF/DRAM source), keep False
OUT_DMA_PSUM = False


def build_nc(s_val: float, n_iters: int = 1, hw_loop: bool = False) -> bass.Bass:
    nc = bass.Bass()

    a_ext = nc.declare_dram_parameter("aT", [E, NT * 128], BF16, isOutput=False)
    x_ext = nc.declare_dram_parameter("xT", [E, B], BF16, isOutput=False)
    xss_ext = nc.declare_dram_parameter("xss", [C_LOC, B], BF16, isOutput=False)
    yt_ext = nc.declare_dram_parameter("yt", [C_LOC, B], BF16, isOutput=False)
    ind_ext = nc.declare_dram_parameter("indc", [128, NG * 256], FP8, isOutput=False)
    indb_ext = nc.declare_dram_parameter("indb", [128, 252], BF16, isOutput=False)
    out_ext = nc.declare_dram_parameter("out", [128, B], BF16, isOutput=True)

    Mult = mybir.AluOpType.mult
    Exp = mybir.ActivationFunctionType.Exp
    Ln = mybir.ActivationFunctionType.Ln
    Square = mybir.ActivationFunctionType.Square
    DR = mybir.MatmulPerfMode.DoubleRow

    with tile.TileContext(nc) as tc:
        with (
            tc.tile_pool(name="persist", bufs=1) as pp,
            tc.tile_pool(name="xload", bufs=2) as p_x,
            tc.tile_pool(name="aload", bufs=2) as p_a,
            tc.tile_pool(name="sq", bufs=16) as p_s,
            tc.tile_pool(name="sqs", bufs=14) as p_ss,
            tc.tile_pool(name="cpb", bufs=4) as p_cp,
            tc.tile_pool(name="fin", bufs=2) as p_f,
            tc.tile_pool(name="psT", bufs=3, space="PSUM") as ps_t,
            tc.tile_pool(name="psQ", bufs=1, space="PSUM") as ps_q,
        ):
            # ---- constants, loaded once
            indc = pp.tile([128, NG * 256], FP8, tag="indc")
            nc.sync.dma_start(out=indc[:], in_=ind_ext[:])
            indb = pp.tile([128, 252], BF16, tag="indb")
            nc.sync.dma_start(out=indb[:], in_=indb_ext[:])
            yt = pp.tile([C_LOC, B], BF16, tag="yt")
            nc.sync.dma_start(out=yt[:], in_=yt_ext[:])
            negs = pp.tile([128, 1], FP32, tag="negs")
            nc.vector.memset(negs[:], -s_val)
            # cross-iteration logits-state buffer; memset so the first
            # (pipelined, discarded) finals pass reads finite values
            qcopy = pp.tile([128, B], BF16, tag="qcopy")
            nc.vector.memset(qcopy[:], 1.0)

            def emit_finals_ops(xss):
                """The 6 pipelined finals ops reading qcopy (prev iter).
                Returns (ops, tail) where tail emits the reduce matmuls +
                output DMA (must be emitted after the last main-loop mm)."""
                q2s = p_f.tile([C_LOC, B], BF16, tag="q2s")
                invs = p_f.tile([C_LOC, B], BF16, tag="invs")
                logitsT = p_f.tile([C_LOC, B], BF16, tag="logitsT")
                expz = p_f.tile([C_LOC, B], BF16, tag="expz")
                tl = p_f.tile([C_LOC, B], BF16, tag="tl")

                ops = [
                    # xss lives at partitions 64:128 so both TT operands
                    # share a base partition (walrus same-base rule)
                    lambda: nc.vector.tensor_tensor(
                        out=q2s[:], in0=qcopy[64:128, :], in1=xss[64:128, :],
                        op=Mult
                    ),
                    lambda: nc.scalar.activation(
                        out=invs[:], in_=q2s[:], func=Ln
                    ),
                    lambda: nc.scalar.activation(
                        out=invs[:], in_=invs[:], func=Exp, scale=-0.5
                    ),
                    lambda: nc.vector.tensor_tensor(
                        out=logitsT[:], in0=qcopy[0:64, :], in1=invs[:], op=Mult
                    ),
                    lambda: nc.scalar.activation(
                        out=expz[:], in_=logitsT[:], func=Exp, scale=s_val,
                        bias=negs[0:C_LOC, :],
                    ),
                    lambda: nc.gpsimd.tensor_tensor(
                        out=tl[:], in0=yt[:], in1=logitsT[:], op=Mult
                    ),
                ]

                def tail():
                    # ship the per-class softmax partials; the host sums the
                    # 64 class rows per shard together with the shard-sum
                    nc.sync.dma_start(out=out_ext[0:C_LOC, :], in_=expz[:])
                    nc.sync.dma_start(out=out_ext[C_LOC:128, :], in_=tl[:])

                return ops, tail

            pipe_state = {}

            def body(pipelined_finals=True):
                prev_sqps = pipe_state.pop("sqps", None)
                # ---- loads (A split in 2 chunks so early tiles start sooner)
                aTb = p_a.tile([128, NT * 128], BF16, tag="aTb")
                half_cols = NT * 64
                nc.sync.dma_start(
                    out=aTb[:, 0:half_cols], in_=a_ext[:, 0:half_cols]
                )
                nc.sync.dma_start(
                    out=aTb[:, half_cols:], in_=a_ext[:, half_cols:]
                )
                xTb = p_x.tile([128, B], BF16, tag="xTb")
                nc.sync.dma_start(out=xTb[:], in_=x_ext[:])
                xss = p_x.tile([128, B], BF16, tag="xss")
                nc.sync.dma_start(out=xss[64:128, :], in_=xss_ext[:])

                fin_ops, fin_tail = emit_finals_ops(xss)
                fin_ops = list(fin_ops) if pipelined_finals else []

                qps = (
                    ps_q.tile([128, B], FP32, tag="q", name="qps")
                    if prev_sqps else None
                )
                tpss, sqps = {}, {}

                def emit_mm(t):
                    tps = ps_t.tile([128, B], FP32, tag="t")
                    asl = aTb[:, t * 128 : (t + 1) * 128]
                    for nb in range(NB):
                        nc.tensor.matmul(
                            tps[:, nb * 512 : (nb + 1) * 512], lhsT=asl,
                            rhs=xTb[:, nb * 512 : (nb + 1) * 512],
                            start=True, stop=True,
                        )
                    tpss[t] = tps

                deferred = {}

                def emit_sq(t):
                    g = t // 2
                    if DR_PAIR[g]:
                        if t % 2 == 0:
                            sqps[g] = p_s.tile(
                                [128, 2 * B], FP8, tag="sqp", name="sqp"
                            )
                        dst = sqps[g][:, (t % 2) * B : (t % 2 + 1) * B]
                    else:
                        dst = p_ss.tile([128, B], BF16, tag="sqs", name="sqs")
                        sqps[("s", t)] = dst
                    if MODE[t] == 'A':
                        nc.scalar.activation(
                            out=dst, in_=tpss[t][:], func=Square
                        )
                    else:
                        cpb = p_cp.tile([128, B], BF16, tag="cpb", name="cpb")
                        nc.vector.tensor_copy(cpb[:], tpss[t][:])
                        if MODE[t] == 'V':
                            deferred.setdefault(t + SQ_LAG, []).append(
                                lambda d=dst, c=cpb: nc.vector.tensor_tensor(
                                    out=d, in0=c[:], in1=c[:], op=Mult
                                )
                            )
                        else:
                            deferred.setdefault(t + SQ_LAG, []).append(
                                lambda d=dst, c=cpb: nc.gpsimd.tensor_tensor(
                                    out=d, in0=c[:], in1=c[:], op=Mult
                                )
                            )
                    del tpss[t]

                def _pair_ap(tile2, nb):
                    # [128 part, 2 k-slots, 512 cols] view of [128, 2B]
                    src = tile2[:, nb * 512 : nb * 512 + 512]
                    return bass.AP(
                        tensor=src.tensor, offset=src.offset,
                        ap=[list(src.ap[0]), [B, 2], [1, 512]],
                    )

                def _ind_ap(g):
                    # [128 part, 2 k-slots, 128 out] slice for pair g
                    src = indc[:, 256 * g : 256 * g + 128]
                    return bass.AP(
                        tensor=src.tensor, offset=src.offset,
                        ap=[list(src.ap[0]), [128, 2], [1, 128]],
                    )

                def emit_red(g, srcs, qdst):
                    for nb in range(NB):
                        if DR_PAIR[g]:
                            nc.tensor.matmul(
                                qdst[:, nb * 512 : (nb + 1) * 512],
                                lhsT=_ind_ap(g),
                                rhs=_pair_ap(srcs[g], nb),
                                start=(g == 0), stop=(g == NG - 1),
                                skip_group_check=True, perf_mode=DR,
                            )
                        else:
                            for i in range(2):
                                t = 2 * g + i
                                off = 4 * g + 64 * i
                                nc.tensor.matmul(
                                    qdst[:, nb * 512 : (nb + 1) * 512],
                                    lhsT=indb[:, 124 - off : 252 - off],
                                    rhs=srcs[("s", t)][:, nb * 512 : (nb + 1) * 512],
                                    start=False,
                                    stop=(g == NG - 1 and i == 1),
                                    skip_group_check=True,
                                )

                # ---- main loop: this body's mm+sq stream, interleaved
                # with the PREVIOUS body's reduces (their square buffers are
                # a full body old, so the PE stream never waits on them)
                for step in range(NT + SQ_LAG):
                    if step < NT:
                        emit_mm(step)
                        emit_sq(step)
                    for fn in deferred.pop(step, ()):
                        fn()
                    if fin_ops and step in FIN_STEPS:
                        fin_ops[FIN_STEPS.index(step)]()
                    if prev_sqps and step % 2 == 1 and step < NT:
                        emit_red(step // 2, prev_sqps, qps)

                # previous-previous iteration's softmax partials + output DMA
                if pipelined_finals:
                    fin_tail()

                # stash the previous iteration's q for its finals
                if prev_sqps:
                    nc.vector.tensor_copy(qcopy[:], qps[:])
                pipe_state["sqps"] = sqps

            def flush_finals():
                # drain the cross-body pipeline: finals for the body whose q
                # is already in qcopy, then reduces + finals for the last body
                xss = p_x.tile([128, B], BF16, tag="xss")
                nc.sync.dma_start(out=xss[64:128, :], in_=xss_ext[:])
                ops, tail = emit_finals_ops(xss)
                for op in ops:
                    op()
                tail()
                sqps = pipe_state.pop("sqps", None)
                if sqps:
                    qps = ps_q.tile([128, B], FP32, tag="q")
                    for g in range(NG):
                        # re-use emit_red structure inline
                        for nb in range(NB):
                            if DR_PAIR[g]:
                                nc.tensor.matmul(
                                    qps[:, nb * 512 : (nb + 1) * 512],
                                    lhsT=bass.AP(
                                        tensor=indc.tensor,
                                        offset=indc[:, 256 * g : 256 * g + 128].offset,
                                        ap=[list(indc.ap[0]), [128, 2], [1, 128]],
                                    ),
                                    rhs=bass.AP(
                                        tensor=sqps[g].tensor,
                                        offset=sqps[g][:, nb * 512 : nb * 512 + 512].offset,
                                        ap=[list(sqps[g].ap[0]), [B, 2], [1, 512]],
                                    ),
                                    start=(g == 0), stop=(g == NG - 1),
                                    skip_group_check=True, perf_mode=DR,
                                )
                            else:
                                for i in range(2):
                                    t = 2 * g + i
                                    off = 4 * g + 64 * i
                                    nc.tensor.matmul(
                                        qps[:, nb * 512 : (nb + 1) * 512],
                                        lhsT=indb[:, 124 - off : 252 - off],
                                        rhs=sqps[("s", t)][:, nb * 512 : (nb + 1) * 512],
                                        start=False,
                                        stop=(g == NG - 1 and i == 1),
                                        skip_group_check=True,
                                    )
                    nc.vector.tensor_copy(qcopy[:], qps[:])
                    xss2 = p_x.tile([128, B], BF16, tag="xss")
                    nc.sync.dma_start(out=xss2[64:128, :], in_=xss_ext[:])
                    ops2, tail2 = emit_finals_ops(xss2)
                    for op in ops2:
                        op()
                    tail2()

            if hw_loop:
                with tc.For_i(0, n_iters, 1):
                    body()
                flush_finals()
            else:
                for _ in range(n_iters):
                    body()
                flush_finals()

    # Split multi-wait sync_info into EventSemaphore instructions (HW allows
    # only 1 wait per instruction in this toolchain's walrus).
    bass_rust.move_matmul_waits_to_ldweights(nc.m)
    bass_rust.generate_event_semaphores(nc)
    return nc


def make_aux():
    # indc[k, 256g + 128*ko + m] = 1 iff (ko=0: m = 4g + k//32)
    #                              or  (ko=1: m = 64 + 4g + k//32)
    indc = np.zeros((128, NG * 256), dtype=ml_dtypes.float8_e4m3)
    for g in range(NG):
        for k in range(128):
            c = k // 32
            indc[k, 256 * g + 4 * g + c] = 1.0
            indc[k, 256 * g + 128 + 64 + 4 * g + c] = 1.0
    # indb[k, 124 + k//32] = 1; window [124-off : 252-off] maps partition k
    # to output row off + k//32 (off = 4g + 64*is_u)
    indb = np.zeros((128, 252), dtype=ml_dtypes.bfloat16)
    for k in range(128):
        indb[k, 124 + k // 32] = 1.0
    return indc, indb


def make_in_maps(x, y, W):
    indc, indb = make_aux()
    xT = np.ascontiguousarray(x.T).astype(ml_dtypes.bfloat16)
    xss_row = np.sum(x.astype(np.float64) ** 2, axis=1)
    xss = np.ascontiguousarray(
        np.broadcast_to(xss_row[None, :], (C_LOC, B))
    ).astype(ml_dtypes.bfloat16)

    nrm = np.linalg.norm(W, axis=2, keepdims=True)
    Wn = (W / np.clip(nrm, 1e-12, None)).astype(np.float64)
    G = Wn @ Wn.transpose(0, 2, 1)                    # (C, S, S)
    L = np.linalg.cholesky(G)
    M = (L.transpose(0, 2, 1) @ Wn).astype(np.float32)  # (C, S, E)
    Wn = Wn.astype(np.float32)

    in_maps = []
    for i in range(NCORES):
        c0 = i * C_LOC
        tiles = []
        for g in range(NG):
            cg = c0 + 4 * g
            tiles.append(Wn[cg : cg + 4].reshape(128, E))
            tiles.append(M[cg : cg + 4].reshape(128, E))
        A = np.concatenate(tiles, axis=0)             # (NT*128, E)
        aT = np.ascontiguousarray(A.T).astype(ml_dtypes.bfloat16)
        yt_i = np.ascontiguousarray(
            y[:, c0 : c0 + C_LOC].T
        ).astype(ml_dtypes.bfloat16)
        in_maps.append(
            {
                "aT": aT, "xT": xT, "xss": xss, "yt": yt_i,
                "indc": indc, "indb": indb,
            }
        )
    return in_maps


def combine(outs, s_val):
    se = np.zeros(B, dtype=np.float64)
    t0 = np.zeros(B, dtype=np.float64)
    for o in outs:
        se += o[0:C_LOC].sum(axis=0)
        t0 += o[C_LOC:128].sum(axis=0)
    return np.float32(np.mean(np.log(se) + s_val - s_val * t0))


_CACHE = {}


def kernel(x, y, W, s, **_unused):
    x = np.ascontiguousarray(np.asarray(x, dtype=np.float32))
    y = np.asarray(y, dtype=np.float32)
    W = np.asarray(W, dtype=np.float32)
    s_val = float(np.asarray(s))

    key = ("v7", s_val)
    nc = _CACHE.get(key)
    if nc is None:
        nc = build_nc(s_val)
        _CACHE[key] = nc

    in_maps = make_in_maps(x, y, W)
    res = run_bass_kernel_spmd(nc, in_maps, core_ids=list(range(NCORES)))
    outs = [np.asarray(r["out"], dtype=np.float64) for r in res.results]
    return combine(outs, s_val)


if __name__ == "__main__":
    rng = np.random.default_rng(0)
    x = rng.standard_normal((B, E), dtype=np.float32)
    lab = rng.integers(0, C, size=B)
    y = np.eye(C, dtype=np.float32)[lab]
    W = rng.uniform(-0.1, 0.1, size=(C, S, E)).astype(np.float32)
    s = np.float32(np.sqrt(2.0) * np.log(C - 1.0))
    print(kernel(x=x, y=y, W=W, s=s))


# revision 25
# speedup vs baseline: 1.4468x; 1.0048x over previous
"""AdaProj loss kernel for 8 TRN2 NeuronCores (Bass/Tile), v7.

Math (per reference):
  xn = l2norm(x, 1); Wn = l2norm(W, 2)  [C,S,E]
  q1 = |Wn_c x|^2 ; q2 = (Wn_c x)^T G_c (Wn_c x), G_c = Wn_c Wn_c^T
  logits = q1/sqrt(q2*|x|^2); loss = mean_b( lse_c(s*logits) - s*logits[b,lab] )

v7: host precomputes Wn and the Cholesky G_c = L_c L_c^T, so
  q2 = |M_c x|^2 with M_c = L_c^T Wn_c.  Both quadratic forms become
  squared linear projections of x. Host stacks A = interleave(Wn, M) into
  32 cs-tiles of 128 rows (even tile 2g: Wn rows of classes 4g..4g+3,
  odd tile 2g+1: M rows of the same classes).

Device per tile t: one matmul t_ps = A_t^T x  [128, B] psum; one fused
square-evacuation psum -> fp8 SBUF (Act activation(Square) or DVE
tensor_tensor mult, alternating for engine balance). Pairs (2g, 2g+1)
land in one [128, 2B] fp8 buffer = (ep | up); a single fp8 DoubleRow
indicator matmul per pair accumulates q1 into psum rows 0:64 and q2
into rows 64:128 of the same psum tile.

|x|^2 comes from the host (xss broadcast [64, B] bf16).

The finals (logits -> softmax partials) are software-pipelined ACROSS
iterations: each body ends with qps -> qcopy (persistent SBUF); the
finals chain reading qcopy is emitted interleaved into the NEXT body's
tile loop (and once after the loop for the last iteration), so the
serial logits tail hides under the next iteration's matmul/square work.

Sharding: class-parallel, C=512 -> 64 classes/core. Each core returns
  out[0,:] = sum_{c in shard} exp(s*logits - s) ; out[1,:] = sum_c y*logits
Host: loss = mean( log(sum_i se_i) + s - s*sum_i t0_i ).
"""

import sys

for _p in ("/opt/trn_rl_repo",):
    if _p not in sys.path:
        sys.path.insert(0, _p)

import ml_dtypes
import numpy as np

import bass_rust
import concourse.bass as bass
import concourse.tile as tile
from concourse import mybir
from concourse.bass_utils import run_bass_kernel_spmd

FP32 = mybir.dt.float32
BF16 = mybir.dt.bfloat16
FP8 = mybir.dt.float8e4

B, C, S, E = 1024, 512, 32, 128
NCORES = 8
C_LOC = C // NCORES            # 64 classes per core
NG = C_LOC // 4                # 16 groups of 4 classes
NT = 2 * NG                    # 32 cs-tiles (even: Wn/q1, odd: M/q2)
NB = B // 512                  # psum-bank chunks of the batch

# square-evac mode per tile:
#   'A': Act activation(Square) psum->fp8 (fused evac+square)
#   'V': DVE copy psum->bf16, then DVE TT square ->fp8 (keeps fp8 DR pair)
#   'P': DVE copy psum->bf16, then Pool TT square ->bf16 (single bf16 reduce)
# (DVE cannot square from PSUM: only one PSUM operand per instruction.)
# per-PAIR modes, interleaved so the Act/DVE/Pool square streams overlap
# (a bunched P-stretch serializes on the slow Pool TT)
import os as _os
_PAIRS = (_os.environ.get("V7_PAIRS") or
          "AA,PP,AA,PP,AA,AV,PP,AA,PP,AA,VV,PP,AA,AA,AA,AA").split(",")
MODE = [m for p in _PAIRS for m in p]
assert len(MODE) == NT
# pairs where both tiles produce fp8 use one DoubleRow reduce; others get
# two single-tile reduces through the sliding-window indicator indb
DR_PAIR = [MODE[2 * g] != 'P' and MODE[2 * g + 1] != 'P' for g in range(NG)]

# tile steps at which the 6 pipelined finals ops are emitted
FIN_STEPS = (6, 10, 14, 18, 22, 26)
# defer the from-SBUF squares (V: DVE, P: Pool) this many steps after the
# evacuating copy, so they never delay the PSUM-freeing copies behind them
SQ_LAG = 3
# DMA the output straight from PSUM (no SBUF bounce) — rejected by this
# toolchain (dma_start requires SBUF/DRAM source), keep False
OUT_DMA_PSUM = False


def build_nc(s_val: float, n_iters: int = 1, hw_loop: bool = False) -> bass.Bass:
    nc = bass.Bass()

    a_ext = nc.declare_dram_parameter("aT", [E, NT * 128], BF16, isOutput=False)
    x_ext = nc.declare_dram_parameter("xT", [E, B], BF16, isOutput=False)
    xss_ext = nc.declare_dram_parameter("xss", [C_LOC, B], BF16, isOutput=False)
    yt_ext = nc.declare_dram_parameter("yt", [C_LOC, B], BF16, isOutput=False)
    ind_ext = nc.declare_dram_parameter("indc", [128, NG * 256], FP8, isOutput=False)
    indb_ext = nc.declare_dram_parameter("indb", [128, 252], BF16, isOutput=False)
    out_ext = nc.declare_dram_parameter("out", [128, B], BF16, isOutput=True)

    Mult = mybir.AluOpType.mult
    Exp = mybir.ActivationFunctionType.Exp
    Ln = mybir.ActivationFunctionType.Ln
    Square = mybir.ActivationFunctionType.Square
    DR = mybir.MatmulPerfMode.DoubleRow

    with tile.TileContext(nc) as tc:
        with (
            tc.tile_pool(name="persist", bufs=1) as pp,
            tc.tile_pool(name="xload", bufs=2) as p_x,
            tc.tile_pool(name="aload", bufs=2) as p_a,
            tc.tile_pool(name="sq", bufs=sum(DR_PAIR) + 5) as p_s,
            tc.tile_pool(name="sqs", bufs=2 * (NG - sum(DR_PAIR)) + 6) as p_ss,
            tc.tile_pool(name="cpb", bufs=4) as p_cp,
            tc.tile_pool(name="fin", bufs=2) as p_f,
            tc.tile_pool(name="psT", bufs=3, space="PSUM") as ps_t,
            tc.tile_pool(name="psQ", bufs=1, space="PSUM") as ps_q,
        ):
            # ---- constants, loaded once
            indc = pp.tile([128, NG * 256], FP8, tag="indc")
            nc.sync.dma_start(out=indc[:], in_=ind_ext[:])
            indb = pp.tile([128, 252], BF16, tag="indb")
            nc.sync.dma_start(out=indb[:], in_=indb_ext[:])
            yt = pp.tile([C_LOC, B], BF16, tag="yt")
            nc.sync.dma_start(out=yt[:], in_=yt_ext[:])
            negs = pp.tile([128, 1], FP32, tag="negs")
            nc.vector.memset(negs[:], -s_val)
            # cross-iteration logits-state buffer; memset so the first
            # (pipelined, discarded) finals pass reads finite values
            qcopy = pp.tile([128, B], BF16, tag="qcopy")
            nc.vector.memset(qcopy[:], 1.0)

            def emit_finals_ops(xss):
                """The 6 pipelined finals ops reading qcopy (prev iter).
                Returns (ops, tail) where tail emits the reduce matmuls +
                output DMA (must be emitted after the last main-loop mm)."""
                q2s = p_f.tile([C_LOC, B], BF16, tag="q2s")
                invs = p_f.tile([C_LOC, B], BF16, tag="invs")
                logitsT = p_f.tile([C_LOC, B], BF16, tag="logitsT")
                expz = p_f.tile([C_LOC, B], BF16, tag="expz")
                tl = p_f.tile([C_LOC, B], BF16, tag="tl")

                ops = [
                    # xss lives at partitions 64:128 so both TT operands
                    # share a base partition (walrus same-base rule)
                    lambda: nc.vector.tensor_tensor(
                        out=q2s[:], in0=qcopy[64:128, :], in1=xss[64:128, :],
                        op=Mult
                    ),
                    lambda: nc.scalar.activation(
                        out=invs[:], in_=q2s[:], func=Ln
                    ),
                    lambda: nc.scalar.activation(
                        out=invs[:], in_=invs[:], func=Exp, scale=-0.5
                    ),
                    lambda: nc.vector.tensor_tensor(
                        out=logitsT[:], in0=qcopy[0:64, :], in1=invs[:], op=Mult
                    ),
                    lambda: nc.scalar.activation(
                        out=expz[:], in_=logitsT[:], func=Exp, scale=s_val,
                        bias=negs[0:C_LOC, :],
                    ),
                    lambda: nc.gpsimd.tensor_tensor(
                        out=tl[:], in0=yt[:], in1=logitsT[:], op=Mult
                    ),
                ]

                def tail():
                    # ship the per-class softmax partials; the host sums the
                    # 64 class rows per shard together with the shard-sum
                    nc.sync.dma_start(out=out_ext[0:C_LOC, :], in_=expz[:])
                    nc.sync.dma_start(out=out_ext[C_LOC:128, :], in_=tl[:])

                return ops, tail

            pipe_state = {}

            def body(pipelined_finals=True):
                prev_sqps = pipe_state.pop("sqps", None)
                # ---- loads (A split in 2 chunks so early tiles start sooner)
                aTb = p_a.tile([128, NT * 128], BF16, tag="aTb")
                half_cols = NT * 64
                nc.sync.dma_start(
                    out=aTb[:, 0:half_cols], in_=a_ext[:, 0:half_cols]
                )
                nc.sync.dma_start(
                    out=aTb[:, half_cols:], in_=a_ext[:, half_cols:]
                )
                xTb = p_x.tile([128, B], BF16, tag="xTb")
                nc.sync.dma_start(out=xTb[:], in_=x_ext[:])
                xss = p_x.tile([128, B], BF16, tag="xss")
                nc.sync.dma_start(out=xss[64:128, :], in_=xss_ext[:])

                fin_ops, fin_tail = emit_finals_ops(xss)
                fin_ops = list(fin_ops) if pipelined_finals else []

                qps = (
                    ps_q.tile([128, B], FP32, tag="q", name="qps")
                    if prev_sqps else None
                )
                tpss, sqps = {}, {}

                def emit_mm(t):
                    tps = ps_t.tile([128, B], FP32, tag="t")
                    asl = aTb[:, t * 128 : (t + 1) * 128]
                    for nb in range(NB):
                        nc.tensor.matmul(
                            tps[:, nb * 512 : (nb + 1) * 512], lhsT=asl,
                            rhs=xTb[:, nb * 512 : (nb + 1) * 512],
                            start=True, stop=True,
                        )
                    tpss[t] = tps

                deferred = {}

                def emit_sq(t):
                    g = t // 2
                    if DR_PAIR[g]:
                        if t % 2 == 0:
                            sqps[g] = p_s.tile(
                                [128, 2 * B], FP8, tag="sqp", name="sqp"
                            )
                        dst = sqps[g][:, (t % 2) * B : (t % 2 + 1) * B]
                    else:
                        dst = p_ss.tile([128, B], BF16, tag="sqs", name="sqs")
                        sqps[("s", t)] = dst
                    if MODE[t] == 'A':
                        nc.scalar.activation(
                            out=dst, in_=tpss[t][:], func=Square
                        )
                    else:
                        cpb = p_cp.tile([128, B], BF16, tag="cpb", name="cpb")
                        nc.vector.tensor_copy(cpb[:], tpss[t][:])
                        if MODE[t] == 'V':
                            deferred.setdefault(t + SQ_LAG, []).append(
                                lambda d=dst, c=cpb: nc.vector.tensor_tensor(
                                    out=d, in0=c[:], in1=c[:], op=Mult
                                )
                            )
                        else:
                            deferred.setdefault(t + SQ_LAG, []).append(
                                lambda d=dst, c=cpb: nc.gpsimd.tensor_tensor(
                                    out=d, in0=c[:], in1=c[:], op=Mult
                                )
                            )
                    del tpss[t]

                def _pair_ap(tile2, nb):
                    # [128 part, 2 k-slots, 512 cols] view of [128, 2B]
                    src = tile2[:, nb * 512 : nb * 512 + 512]
                    return bass.AP(
                        tensor=src.tensor, offset=src.offset,
                        ap=[list(src.ap[0]), [B, 2], [1, 512]],
                    )

                def _ind_ap(g):
                    # [128 part, 2 k-slots, 128 out] slice for pair g
                    src = indc[:, 256 * g : 256 * g + 128]
                    return bass.AP(
                        tensor=src.tensor, offset=src.offset,
                        ap=[list(src.ap[0]), [128, 2], [1, 128]],
                    )

                def emit_red(g, srcs, qdst):
                    for nb in range(NB):
                        if DR_PAIR[g]:
                            nc.tensor.matmul(
                                qdst[:, nb * 512 : (nb + 1) * 512],
                                lhsT=_ind_ap(g),
                                rhs=_pair_ap(srcs[g], nb),
                                start=(g == 0), stop=(g == NG - 1),
                                skip_group_check=True, perf_mode=DR,
                            )
                        else:
                            for i in range(2):
                                t = 2 * g + i
                                off = 4 * g + 64 * i
                                nc.tensor.matmul(
                                    qdst[:, nb * 512 : (nb + 1) * 512],
                                    lhsT=indb[:, 124 - off : 252 - off],
                                    rhs=srcs[("s", t)][:, nb * 512 : (nb + 1) * 512],
                                    start=False,
                                    stop=(g == NG - 1 and i == 1),
                                    skip_group_check=True,
                                )

                # ---- main loop: this body's mm+sq stream, interleaved
                # with the PREVIOUS body's reduces (their square buffers are
                # a full body old, so the PE stream never waits on them)
                for step in range(NT + SQ_LAG):
                    if step < NT:
                        emit_mm(step)
                        emit_sq(step)
                    for fn in deferred.pop(step, ()):
                        fn()
                    if fin_ops and step in FIN_STEPS:
                        fin_ops[FIN_STEPS.index(step)]()
                    if prev_sqps and step % 2 == 1 and step < NT:
                        emit_red(step // 2, prev_sqps, qps)

                # previous-previous iteration's softmax partials + output DMA
                if pipelined_finals:
                    fin_tail()

                # stash the previous iteration's q for its finals
                if prev_sqps:
                    nc.vector.tensor_copy(qcopy[:], qps[:])
                pipe_state["sqps"] = sqps

            def flush_finals():
                # drain the cross-body pipeline: finals for the body whose q
                # is already in qcopy, then reduces + finals for the last body
                xss = p_x.tile([128, B], BF16, tag="xss")
                nc.sync.dma_start(out=xss[64:128, :], in_=xss_ext[:])
                ops, tail = emit_finals_ops(xss)
                for op in ops:
                    op()
                tail()
                sqps = pipe_state.pop("sqps", None)
                if sqps:
                    qps = ps_q.tile([128, B], FP32, tag="q")
                    for g in range(NG):
                        # re-use emit_red structure inline
                        for nb in range(NB):
                            if DR_PAIR[g]:
                                nc.tensor.matmul(
                                    qps[:, nb * 512 : (nb + 1) * 512],
                                    lhsT=bass.AP(
                                        tensor=indc.tensor,
                                        offset=indc[:, 256 * g : 256 * g + 128].offset,
                                        ap=[list(indc.ap[0]), [128, 2], [1, 128]],
                                    ),
                                    rhs=bass.AP(
                                        tensor=sqps[g].tensor,
                                        offset=sqps[g][:, nb * 512 : nb * 512 + 512].offset,
                                        ap=[list(sqps[g].ap[0]), [B, 2], [1, 512]],
                                    ),
                                    start=(g == 0), stop=(g == NG - 1),
                                    skip_group_check=True, perf_mode=DR,
                                )
                            else:
                                for i in range(2):
                                    t = 2 * g + i
                                    off = 4 * g + 64 * i
                                    nc.tensor.matmul(
                                        qps[:, nb * 512 : (nb + 1) * 512],
                                        lhsT=indb[:, 124 - off : 252 - off],
                                        rhs=sqps[("s", t)][:, nb * 512 : (nb + 1) * 512],
                                        start=False,
                                        stop=(g == NG - 1 and i == 1),
                                        skip_group_check=True,
                                    )
                    nc.vector.tensor_copy(qcopy[:], qps[:])
                    xss2 = p_x.tile([128, B], BF16, tag="xss")
                    nc.sync.dma_start(out=xss2[64:128, :], in_=xss_ext[:])
                    ops2, tail2 = emit_finals_ops(xss2)
                    for op in ops2:
                        op()
                    tail2()

            if hw_loop:
                with tc.For_i(0, n_iters, 1):
                    body()
                flush_finals()
            else:
                for _ in range(n_iters):
                    body()
                flush_finals()

    # Split multi-wait sync_info into EventSemaphore instructions (HW allows
    # only 1 wait per instruction in this toolchain's walrus).
    bass_rust.move_matmul_waits_to_ldweights(nc.m)
    bass_rust.generate_event_semaphores(nc)
    return nc


def make_aux():
    # indc[k, 256g + 128*ko + m] = 1 iff (ko=0: m = 4g + k//32)
    #                              or  (ko=1: m = 64 + 4g + k//32)
    indc = np.zeros((128, NG * 256), dtype=ml_dtypes.float8_e4m3)
    for g in range(NG):
        for k in range(128):
            c = k // 32
            indc[k, 256 * g + 4 * g + c] = 1.0
            indc[k, 256 * g + 128 + 64 + 4 * g + c] = 1.0
    # indb[k, 124 + k//32] = 1; window [124-off : 252-off] maps partition k
    # to output row off + k//32 (off = 4g + 64*is_u)
    indb = np.zeros((128, 252), dtype=ml_dtypes.bfloat16)
    for k in range(128):
        indb[k, 124 + k // 32] = 1.0
    return indc, indb


def make_in_maps(x, y, W):
    indc, indb = make_aux()
    xT = np.ascontiguousarray(x.T).astype(ml_dtypes.bfloat16)
    xss_row = np.sum(x.astype(np.float64) ** 2, axis=1)
    xss = np.ascontiguousarray(
        np.broadcast_to(xss_row[None, :], (C_LOC, B))
    ).astype(ml_dtypes.bfloat16)

    nrm = np.linalg.norm(W, axis=2, keepdims=True)
    Wn = (W / np.clip(nrm, 1e-12, None)).astype(np.float64)
    G = Wn @ Wn.transpose(0, 2, 1)                    # (C, S, S)
    L = np.linalg.cholesky(G)
    M = (L.transpose(0, 2, 1) @ Wn).astype(np.float32)  # (C, S, E)
    Wn = Wn.astype(np.float32)

    in_maps = []
    for i in range(NCORES):
        c0 = i * C_LOC
        tiles = []
        for g in range(NG):
            cg = c0 + 4 * g
            tiles.append(Wn[cg : cg + 4].reshape(128, E))
            tiles.append(M[cg : cg + 4].reshape(128, E))
        A = np.concatenate(tiles, axis=0)             # (NT*128, E)
        aT = np.ascontiguousarray(A.T).astype(ml_dtypes.bfloat16)
        yt_i = np.ascontiguousarray(
            y[:, c0 : c0 + C_LOC].T
        ).astype(ml_dtypes.bfloat16)
        in_maps.append(
            {
                "aT": aT, "xT": xT, "xss": xss, "yt": yt_i,
                "indc": indc, "indb": indb,
            }
        )
    return in_maps


def combine(outs, s_val):
    se = np.zeros(B, dtype=np.float64)
    t0 = np.zeros(B, dtype=np.float64)
    for o in outs:
        se += o[0:C_LOC].sum(axis=0)
        t0 += o[C_LOC:128].sum(axis=0)
    return np.float32(np.mean(np.log(se) + s_val - s_val * t0))


_CACHE = {}


def kernel(x, y, W, s, **_unused):
    x = np.ascontiguousarray(np.asarray(x, dtype=np.float32))
    y = np.asarray(y, dtype=np.float32)
    W = np.asarray(W, dtype=np.float32)
    s_val = float(np.asarray(s))

    key = ("v7", s_val)
    nc = _CACHE.get(key)
    if nc is None:
        nc = build_nc(s_val)
        _CACHE[key] = nc

    in_maps = make_in_maps(x, y, W)
    res = run_bass_kernel_spmd(nc, in_maps, core_ids=list(range(NCORES)))
    outs = [np.asarray(r["out"], dtype=np.float64) for r in res.results]
    return combine(outs, s_val)


if __name__ == "__main__":
    rng = np.random.default_rng(0)
    x = rng.standard_normal((B, E), dtype=np.float32)
    lab = rng.integers(0, C, size=B)
    y = np.eye(C, dtype=np.float32)[lab]
    W = rng.uniform(-0.1, 0.1, size=(C, S, E)).astype(np.float32)
    s = np.float32(np.sqrt(2.0) * np.log(C - 1.0))
    print(kernel(x=x, y=y, W=W, s=s))
